# revision 1
# baseline (speedup 1.0000x reference)
"""Trainium2 Bass kernel for nn_FCGF_MLP2 (ragged segment max-pool -> 1x1 conv
-> BatchNorm(train) -> ReLU), SPMD across 8 NeuronCores.

Strategy
--------
Segments (4096, ragged lengths 312..712) are globally sorted by length
(descending) and cut into 4 "bands" of 1024 ranks each.  Band j is padded to a
single static length L[j] (= the band's max, rounded up to a multiple of 8),
so every (core, band) tile is a dense [128 segments, 32 ch, L[j]] block --
raggedness is absorbed into ~10% padding with fp16 -max.

x is staged to HBM in fp16: the kernel is memory-bound and fp16 halves the
stream; quantizing x before the max is exact up to one fp16 rounding of the
max element (~2e-3 final rel err after BN, vs the 2e-2 gate).

Per core:
  * partition = segment, row = [8 ch, L[j]] per chunk.  The segment max runs
    on the DVE as a 3-level pairwise fp16 max tree (tensor_tensor) followed
    by a plain reduce_max over the remaining L/8.  (A flat-2D layout would
    let the tree hit the DVE 2x 16-bit mode -- 44 vs 70 us isolated -- but
    measured end-to-end it LOSES to this blocked form, which overlaps the
    DMA stream better: 78.4 vs 82.8-91.7 us.  InstTensorReduce itself has
    no fast mode at all, so the tree still halves the plain-reduce cost.)
  * per band: one PE transpose of pooled [128,32] -> PSUM, an ACT-engine copy
    to SBUF, one K=32 matmul into y_ps, then bias + per-band partial BN stats
    (sum via Identity+accum, sumsq via Square+accum, both on ACT).
  * AllReduce of [128, 2] stats over 8 cores -> global mean/var.
  * y_norm = relu(y * scale + shift) with per-partition scale/shift
    -> PE transpose back -> [512, 128] output (sorted order; host unpermutes).
"""

import numpy as np

B = 4096
N = B * 512
C_IN = 32
C_OUT = 128
N_CORES = 8
P = 128                       # partitions / segments per tile
N_BANDS = 4                   # tiles per core
SEGS_PER_CORE = P * N_BANDS   # 512
BN_EPS = 1e-5
CH = 8                        # channels per DMA/reduce chunk (32 % CH == 0)
FMIN = np.float16(np.finfo(np.float16).min)  # x is staged in fp16

_prog_cache = {}


def _build_program(Ls, no_cc=False, repeat=1, unroll=1):
    """Trace the SPMD bass program for static band lengths Ls (len N_BANDS).

    no_cc=True skips the AllReduce (local BN stats) — used for timing.
    repeat>1 wraps the body in a hardware loop (timing use only).
    unroll>1 traces the body multiple times (TimelineSim steady-state use).
    """
    from contextlib import ExitStack

    import concourse.bacc as bacc
    import concourse.mybir as mybir
    import concourse.tile as tile
    from concourse.masks import make_identity

    f32 = mybir.dt.float32
    f16 = mybir.dt.float16
    Ltot = int(sum(Ls))
    n_cc = C_IN // CH

    nc = bacc.Bacc(None, num_devices=N_CORES)
    xp = nc.dram_tensor("xp", [P, 32 * Ltot], f16, kind="ExternalInput")
    wt = nc.dram_tensor("wt", [C_IN, C_OUT], f32, kind="ExternalInput")
    cb = nc.dram_tensor("cb", [C_OUT, 1], f32, kind="ExternalInput")
    gm = nc.dram_tensor("gm", [C_OUT, 1], f32, kind="ExternalInput")
    bt = nc.dram_tensor("bt", [C_OUT, 1], f32, kind="ExternalInput")
    out = nc.dram_tensor("out", [SEGS_PER_CORE, C_OUT], f32, kind="ExternalOutput")

    with tile.TileContext(nc) as tc, ExitStack() as ctx:
        singles = ctx.enter_context(tc.tile_pool(name="singles", bufs=1))
        xpool = ctx.enter_context(tc.tile_pool(name="x", bufs=4))
        m1p = ctx.enter_context(tc.tile_pool(name="m1", bufs=3))
        m2p = ctx.enter_context(tc.tile_pool(name="m2", bufs=3))
        m3p = ctx.enter_context(tc.tile_pool(name="m3", bufs=3))
        m4p = ctx.enter_context(tc.tile_pool(name="m4", bufs=3))
        ppool = ctx.enter_context(tc.tile_pool(name="pooled", bufs=2))
        gpool = ctx.enter_context(tc.tile_pool(name="ptg", bufs=2))
        spool = ctx.enter_context(tc.tile_pool(name="small", bufs=2))
        opool = ctx.enter_context(tc.tile_pool(name="outs", bufs=2))
        ofast = ctx.enter_context(tc.tile_pool(name="ofast", bufs=2))
        ps_tp = ctx.enter_context(tc.tile_pool(name="ps_tp", bufs=2, space="PSUM"))
        ps_tp2 = ctx.enter_context(tc.tile_pool(name="ps_tp2", bufs=4, space="PSUM"))
        ps_y = ctx.enter_context(tc.tile_pool(name="ps_y", bufs=2, space="PSUM"))
        dram = ctx.enter_context(tc.tile_pool(name="dram", bufs=2, space="DRAM"))

        # constants (outside the repeat loop)
        wt_sb = singles.tile([C_IN, C_OUT], f32)
        nc.gpsimd.dma_start(out=wt_sb[:], in_=wt[:])
        cb_sb = singles.tile([C_OUT, 1], f32)
        nc.gpsimd.dma_start(out=cb_sb[:], in_=cb[:])
        gm_sb = singles.tile([C_OUT, 1], f32)
        nc.gpsimd.dma_start(out=gm_sb[:], in_=gm[:])
        bt_sb = singles.tile([C_OUT, 1], f32)
        nc.gpsimd.dma_start(out=bt_sb[:], in_=bt[:])
        ident = singles.tile([P, P], f32)
        make_identity(nc, ident[:])
        eps_sb = singles.tile([P, 1], f32)
        nc.vector.memset(eps_sb[:], BN_EPS)
        # warm the ACT function table (all five funcs share one set; loading
        # it up front keeps the ~1.3us table load off the critical path)
        warm = singles.tile([P, 1], f32)
        for fn in ("Sqrt", "Relu", "Copy", "Identity", "Square"):
            nc.scalar.activation(
                out=warm[:], in_=eps_sb[:],
                func=getattr(mybir.ActivationFunctionType, fn),
            )

        def body():
            # ---- segment max-pool over bands + per-band conv/stats ----
            y_sb = opool.tile([C_OUT, SEGS_PER_CORE], f32, tag="y")
            sums = spool.tile([P, N_BANDS], f32, tag="sums")
            sqs = spool.tile([P, N_BANDS], f32, tag="sqs")
            ysq = opool.tile([C_OUT, P], f32, tag="ysq")
            off = 0
            for j in range(N_BANDS):
                Lj = int(Ls[j])
                pooled_j = ppool.tile([P, C_IN], f32, tag="pooled")
                h, q, e = Lj // 2, Lj // 4, Lj // 8
                for i in range(n_cc):
                    xt = xpool.tile([P, CH, Lj], f16, tag="xt")
                    base = 32 * off + i * CH * Lj
                    nc.sync.dma_start(out=xt[:], in_=xp[:, base : base + CH * Lj])
                    # pairwise fp16 max tree, then a short 1x reduce: three
                    # halving tensor_tensor levels leave only L/8 for the
                    # reduce pass (InstTensorReduce has no fast mode)
                    m1 = m1p.tile([P, CH, h], f16, tag="m1")
                    nc.vector.tensor_tensor(
                        out=m1[:], in0=xt[:, :, 0:h], in1=xt[:, :, h:Lj],
                        op=mybir.AluOpType.max)
                    m2 = m2p.tile([P, CH, q], f16, tag="m2")
                    nc.vector.tensor_tensor(
                        out=m2[:], in0=m1[:, :, 0:q], in1=m1[:, :, q:h],
                        op=mybir.AluOpType.max)
                    m3 = m3p.tile([P, CH, e], f16, tag="m3")
                    nc.vector.tensor_tensor(
                        out=m3[:], in0=m2[:, :, 0:e], in1=m2[:, :, e:q],
                        op=mybir.AluOpType.max)
                    nc.vector.reduce_max(
                        out=pooled_j[:, i * CH : (i + 1) * CH],
                        in_=m3[:], axis=mybir.AxisListType.X)
                # band conv: transpose pooled -> [32, P], copy to SBUF (ACT),
                # one K=32 matmul, then bias + partial BN stats (ACT)
                tp = ps_tp.tile([C_IN, P], f32, tag="tp")
                nc.tensor.transpose(tp[:], pooled_j[:], ident[:])
                ptg = gpool.tile([C_IN, P], f32, tag="ptg")
                nc.scalar.copy(out=ptg[:], in_=tp[:])
                y_ps = ps_y.tile([C_OUT, P], f32, tag="yps")
                nc.tensor.matmul(y_ps[:], wt_sb[:], ptg[:], start=True, stop=True)
                ycol = y_sb[:, j * P : (j + 1) * P]
                nc.scalar.activation(
                    out=ycol, in_=y_ps[:],
                    func=mybir.ActivationFunctionType.Identity,
                    bias=cb_sb[:], scale=1.0,
                    accum_out=sums[:, j : j + 1],
                )
                nc.scalar.activation(
                    out=ysq[:], in_=ycol,
                    func=mybir.ActivationFunctionType.Square,
                    accum_out=sqs[:, j : j + 1],
                )
                off += Lj

            # combine band partials -> [128, 2]
            stats = spool.tile([P, 2], f32, tag="stats")
            nc.vector.reduce_sum(out=stats[:, 0:1], in_=sums[:],
                                 axis=mybir.AxisListType.X)
            nc.vector.reduce_sum(out=stats[:, 1:2], in_=sqs[:],
                                 axis=mybir.AxisListType.X)

            # ---- AllReduce stats over the 8 cores ----
            if no_cc:
                gstats = stats
            else:
                cc_in = dram.tile([P, 2], f32, tag="ccin")
                cc_out = dram.tile([P, 2], f32, tag="ccout")
                nc.gpsimd.dma_start(out=cc_in[:], in_=stats[:])
                nc.gpsimd.collective_compute(
                    "AllReduce",
                    mybir.AluOpType.add,
                    replica_groups=[list(range(N_CORES))],
                    ins=[cc_in.opt()],
                    outs=[cc_out.opt()],
                )
                gstats = spool.tile([P, 2], f32, tag="gstats")
                nc.gpsimd.dma_start(out=gstats[:], in_=cc_out[:])

            # ---- BN scale/shift ----
            me = spool.tile([P, 2], f32, tag="me")
            nc.scalar.mul(out=me[:], in_=gstats[:], mul=1.0 / B)
            mean = me[:, 0:1]
            var = spool.tile([P, 1], f32, tag="var")
            nc.vector.tensor_mul(out=var[:], in0=mean, in1=mean)
            nc.vector.tensor_sub(out=var[:], in0=me[:, 1:2], in1=var[:])
            std = spool.tile([P, 1], f32, tag="std")
            nc.scalar.activation(
                out=std[:], in_=var[:],
                func=mybir.ActivationFunctionType.Sqrt,
                bias=eps_sb[:])
            rstd = spool.tile([P, 1], f32, tag="rstd")
            nc.vector.reciprocal(out=rstd[:], in_=std[:])
            scl = spool.tile([P, 1], f32, tag="scl")
            nc.vector.tensor_mul(out=scl[:], in0=gm_sb[:], in1=rstd[:])
            shf = spool.tile([P, 1], f32, tag="shf")
            nc.vector.tensor_mul(out=shf[:], in0=mean, in1=scl[:])
            nc.vector.tensor_sub(out=shf[:], in0=bt_sb[:], in1=shf[:])

            # ---- normalize + relu (per band), transpose back, one store ----
            o_sb = ofast.tile([P, N_BANDS, C_OUT], f32, tag="o")
            for j in range(N_BANDS):
                yf = opool.tile([C_OUT, P], f32, tag="yf")
                nc.scalar.activation(
                    out=yf[:], in_=y_sb[:, j * P : (j + 1) * P],
                    func=mybir.ActivationFunctionType.Relu,
                    bias=shf[:], scale=scl[:],
                )
                tp2 = ps_tp2.tile([P, P], f32, tag="tp2")
                nc.tensor.transpose(tp2[:], yf[:], ident[:])
                nc.vector.tensor_copy(o_sb[:, j, :], tp2[:])
            # out[j*P + p, c] <- o_sb[p, j, c]; two DMAs so the first half's
            # transfer overlaps the second half's transposes/copies
            out_view = out.rearrange("(j p) c -> p j c", p=P)
            nc.sync.dma_start(out=out_view[:, 0:2, :], in_=o_sb[:, 0:2, :])
            nc.sync.dma_start(out=out_view[:, 2:4, :], in_=o_sb[:, 2:4, :])

        if repeat > 1:
            with tc.For_i(0, repeat, 1):
                for _ in range(unroll):
                    body()
        else:
            for _ in range(unroll):
                body()

    nc.compile()
    return nc


def _layout(length):
    """Global sort -> band lengths (ceil to 8), per-(core,band) segment ids."""
    length = np.asarray(length, np.int64)
    starts = np.zeros(B, np.int64)
    starts[1:] = np.cumsum(length)[:-1]
    order = np.argsort(-length, kind="stable")
    band = N_CORES * P
    # multiple of 8 so each chunk supports 3 clean halvings
    Ls = [-(-int(length[order[band * j]]) // 8) * 8 for j in range(N_BANDS)]
    # seg_ids[c, j, p] = original segment id handled by core c, band j, row p
    seg_ids = np.empty((N_CORES, N_BANDS, P), np.int64)
    for j in range(N_BANDS):
        for c in range(N_CORES):
            seg_ids[c, j] = order[band * j + P * c : band * j + P * (c + 1)]
    return starts, Ls, seg_ids


def _pack_inputs(x, length, conv_w, conv_b, gamma, beta, starts, Ls, seg_ids):
    Ltot = int(sum(Ls))
    xp = np.full((N_CORES, P, 32 * Ltot), FMIN, np.float16)
    offs = np.concatenate([[0], np.cumsum(Ls)]).astype(np.int64)
    length = np.asarray(length, np.int64)
    x = np.asarray(x, np.float32)
    for c in range(N_CORES):
        for j in range(N_BANDS):
            Lj = Ls[j]
            base = 32 * int(offs[j])
            view = xp[c, :, base : base + 32 * Lj].reshape(P, 32, Lj)
            for p in range(P):
                s = int(starts[seg_ids[c, j, p]])
                l = int(length[seg_ids[c, j, p]])
                view[p, :, :l] = x[s : s + l].T
    wt = np.ascontiguousarray(np.asarray(conv_w, np.float32).T)  # [32, 128]
    cb = np.ascontiguousarray(conv_b.reshape(C_OUT, 1), np.float32)
    gm = np.ascontiguousarray(gamma.reshape(C_OUT, 1), np.float32)
    bt = np.ascontiguousarray(beta.reshape(C_OUT, 1), np.float32)
    in_maps = [
        {"xp": xp[c], "wt": wt, "cb": cb, "gm": gm, "bt": bt}
        for c in range(N_CORES)
    ]
    return in_maps


def _run(x, length, conv_w, conv_b, gamma, beta, trace=False):
    from concourse.bass_utils import run_bass_kernel_spmd

    x = np.asarray(x, np.float32)
    length = np.asarray(length)
    assert x.shape == (N, C_IN) and length.shape == (B,)

    starts, Ls, seg_ids = _layout(length)
    in_maps = _pack_inputs(
        x, length, np.asarray(conv_w), np.asarray(conv_b),
        np.asarray(gamma), np.asarray(beta), starts, Ls, seg_ids,
    )

    key = tuple(Ls)
    if key not in _prog_cache:
        _prog_cache[key] = _build_program(Ls)
    nc = _prog_cache[key]

    res = run_bass_kernel_spmd(nc, in_maps, list(range(N_CORES)), trace=trace)

    full = np.empty((B, C_OUT), np.float32)
    for c in range(N_CORES):
        full[seg_ids[c].reshape(-1)] = res.results[c]["out"]
    return full, res


def kernel(x, length, conv_w, conv_b, gamma, beta):
    full, _ = _run(x, length, conv_w, conv_b, gamma, beta, trace=False)
    return full



# revision 2
# speedup vs baseline: 1.0526x; 1.0526x over previous
"""Trainium2 Bass kernel for nn_FCGF_MLP2 (ragged segment max-pool -> 1x1 conv
-> BatchNorm(train) -> ReLU), SPMD across 8 NeuronCores.

Strategy
--------
Segments (4096, ragged lengths 312..712) are globally sorted by length
(descending) and cut into 4 "bands" of 1024 ranks each.  Band j is padded to a
single static length L[j] (= the band's max, rounded up to a multiple of 8),
so every (core, band) tile is a dense [128 segments, 32 ch, L[j]] block --
raggedness is absorbed into ~10% padding with fp16 -max.

x is staged to HBM in fp16: the kernel is memory-bound and fp16 halves the
stream; quantizing x before the max is exact up to one fp16 rounding of the
max element (~2e-3 final rel err after BN, vs the 2e-2 gate).

Per core, flat-2D tree layout: each band row is packed host-side in
tree-slot-major order -- chunks of [2 slots][2][2][32 ch][S cols] fp16 -- so
the 3-level pairwise max tree runs as tensor_tensor on plain 2D contiguous
slices ([P, 128S] -> [P, 64S] -> [P, 32S]).  Contiguous step-1 fp16 operands
hit the DVE 2x_1P perf mode (the previous blocked [P, 8ch, L] form fell back
to 1x on silicon: its 3D access pattern / odd-offset channel rows fail the
packed-mode alignment check).  The remaining [P, 32, S] reduce_max (no fast
mode exists for InstTensorReduce) is only L/8 of the stream.  This halves DVE
busy time (~70us -> ~42us), putting the kernel at the HBM roofline
(~18.4MB/core / ~358GB/s ~= 51us).

Per band: one PE transpose of pooled [128,32] -> PSUM, an ACT-engine copy to
SBUF, one K=32 matmul into y_ps, then bias + per-band partial BN stats (sum
via Identity+accum, sumsq via Square+accum, both on ACT).  AllReduce of
[128, 2] stats over 8 cores -> global mean/var.  y_norm = relu(y * scale +
shift) with per-partition scale/shift -> PE transpose back -> [512, 128]
output (sorted order; host unpermutes).
"""

import numpy as np

B = 4096
N = B * 512
C_IN = 32
C_OUT = 128
N_CORES = 8
P = 128                       # partitions / segments per tile
N_BANDS = 4                   # tiles per core
SEGS_PER_CORE = P * N_BANDS   # 512
BN_EPS = 1e-5
K_CHUNKS = 3                  # DMA/compute chunks per band
FMIN = np.float16(np.finfo(np.float16).min)  # x is staged in fp16

_prog_cache = {}


def _chunk_sizes(G, k=K_CHUNKS):
    """Split G columns into k near-equal chunk sizes."""
    base = G // k
    sizes = [base + (1 if i < G % k else 0) for i in range(k)]
    return [s for s in sizes if s > 0]


def _build_program(Ls, no_cc=False, repeat=1, unroll=1):
    """Trace the SPMD bass program for static band lengths Ls (len N_BANDS).

    no_cc=True skips the AllReduce (local BN stats) — used for timing.
    repeat>1 wraps the body in a hardware loop (timing use only).
    unroll>1 traces the body multiple times (TimelineSim steady-state use).
    """
    from contextlib import ExitStack

    import concourse.bacc as bacc
    import concourse.mybir as mybir
    import concourse.tile as tile
    from concourse.masks import make_identity

    f32 = mybir.dt.float32
    f16 = mybir.dt.float16
    Ltot = int(sum(Ls))

    nc = bacc.Bacc(None, num_devices=N_CORES)
    xp = nc.dram_tensor("xp", [P, 32 * Ltot], f16, kind="ExternalInput")
    wt = nc.dram_tensor("wt", [C_IN, C_OUT], f32, kind="ExternalInput")
    cb = nc.dram_tensor("cb", [C_OUT, 1], f32, kind="ExternalInput")
    gm = nc.dram_tensor("gm", [C_OUT, 1], f32, kind="ExternalInput")
    bt = nc.dram_tensor("bt", [C_OUT, 1], f32, kind="ExternalInput")
    out = nc.dram_tensor("out", [SEGS_PER_CORE, C_OUT], f32, kind="ExternalOutput")

    with tile.TileContext(nc) as tc, ExitStack() as ctx:
        singles = ctx.enter_context(tc.tile_pool(name="singles", bufs=1))
        xpool = ctx.enter_context(tc.tile_pool(name="x", bufs=4))
        m1p = ctx.enter_context(tc.tile_pool(name="m1", bufs=3))
        m2p = ctx.enter_context(tc.tile_pool(name="m2", bufs=3))
        m3p = ctx.enter_context(tc.tile_pool(name="m3", bufs=3))
        ppool = ctx.enter_context(tc.tile_pool(name="pooled", bufs=2))
        gpool = ctx.enter_context(tc.tile_pool(name="ptg", bufs=2))
        spool = ctx.enter_context(tc.tile_pool(name="small", bufs=2))
        opool = ctx.enter_context(tc.tile_pool(name="outs", bufs=2))
        ofast = ctx.enter_context(tc.tile_pool(name="ofast", bufs=2))
        ps_tp = ctx.enter_context(tc.tile_pool(name="ps_tp", bufs=2, space="PSUM"))
        ps_tp2 = ctx.enter_context(tc.tile_pool(name="ps_tp2", bufs=4, space="PSUM"))
        ps_y = ctx.enter_context(tc.tile_pool(name="ps_y", bufs=2, space="PSUM"))
        dram = ctx.enter_context(tc.tile_pool(name="dram", bufs=2, space="DRAM"))

        # constants (outside the repeat loop)
        wt_sb = singles.tile([C_IN, C_OUT], f32)
        nc.gpsimd.dma_start(out=wt_sb[:], in_=wt[:])
        cb_sb = singles.tile([C_OUT, 1], f32)
        nc.gpsimd.dma_start(out=cb_sb[:], in_=cb[:])
        gm_sb = singles.tile([C_OUT, 1], f32)
        nc.gpsimd.dma_start(out=gm_sb[:], in_=gm[:])
        bt_sb = singles.tile([C_OUT, 1], f32)
        nc.gpsimd.dma_start(out=bt_sb[:], in_=bt[:])
        ident = singles.tile([P, P], f32)
        make_identity(nc, ident[:])
        eps_sb = singles.tile([P, 1], f32)
        nc.vector.memset(eps_sb[:], BN_EPS)
        # warm the ACT function table (all five funcs share one set; loading
        # it up front keeps the ~1.3us table load off the critical path)
        warm = singles.tile([P, 1], f32)
        for fn in ("Sqrt", "Relu", "Copy", "Identity", "Square"):
            nc.scalar.activation(
                out=warm[:], in_=eps_sb[:],
                func=getattr(mybir.ActivationFunctionType, fn),
            )

        def body():
            # ---- segment max-pool over bands + per-band conv/stats ----
            y_sb = opool.tile([C_OUT, SEGS_PER_CORE], f32, tag="y")
            sums = spool.tile([P, N_BANDS], f32, tag="sums")
            sqs = spool.tile([P, N_BANDS], f32, tag="sqs")
            ysq = opool.tile([C_OUT, P], f32, tag="ysq")
            off = 0
            for j in range(N_BANDS):
                Lj = int(Ls[j])
                G = Lj // 8
                sizes = _chunk_sizes(G)
                pband = ppool.tile([P, C_IN, len(sizes)], f32, tag="pband")
                base = 32 * off
                for k, S in enumerate(sizes):
                    E = 256 * S  # fp16 elems per partition in this chunk
                    xt = xpool.tile([P, E], f16, tag="xt")
                    nc.sync.dma_start(out=xt[:], in_=xp[:, base : base + E])
                    base += E
                    # 3-level pairwise fp16 max tree on contiguous 2D slices
                    # (DVE 2x_1P), then one short reduce over [P, 32, S].
                    m1 = m1p.tile([P, E // 2], f16, tag="m1")
                    nc.vector.tensor_tensor(
                        out=m1[:], in0=xt[:, 0 : E // 2], in1=xt[:, E // 2 : E],
                        op=mybir.AluOpType.max)
                    m2 = m2p.tile([P, E // 4], f16, tag="m2")
                    nc.vector.tensor_tensor(
                        out=m2[:], in0=m1[:, 0 : E // 4], in1=m1[:, E // 4 : E // 2],
                        op=mybir.AluOpType.max)
                    m3 = m3p.tile([P, C_IN, S], f16, tag="m3")
                    m3f = m3[:].rearrange("p c s -> p (c s)")
                    nc.vector.tensor_tensor(
                        out=m3f, in0=m2[:, 0 : E // 8], in1=m2[:, E // 8 : E // 4],
                        op=mybir.AluOpType.max)
                    nc.vector.reduce_max(
                        out=pband[:, :, k : k + 1],
                        in_=m3[:], axis=mybir.AxisListType.X)
                pooled_j = ppool.tile([P, C_IN], f32, tag="pooled")
                nc.vector.reduce_max(
                    out=pooled_j[:], in_=pband[:], axis=mybir.AxisListType.X)
                # band conv: transpose pooled -> [32, P], copy to SBUF (ACT),
                # one K=32 matmul, then bias + partial BN stats (ACT)
                tp = ps_tp.tile([C_IN, P], f32, tag="tp")
                nc.tensor.transpose(tp[:], pooled_j[:], ident[:])
                ptg = gpool.tile([C_IN, P], f32, tag="ptg")
                nc.scalar.copy(out=ptg[:], in_=tp[:])
                y_ps = ps_y.tile([C_OUT, P], f32, tag="yps")
                nc.tensor.matmul(y_ps[:], wt_sb[:], ptg[:], start=True, stop=True)
                ycol = y_sb[:, j * P : (j + 1) * P]
                nc.scalar.activation(
                    out=ycol, in_=y_ps[:],
                    func=mybir.ActivationFunctionType.Identity,
                    bias=cb_sb[:], scale=1.0,
                    accum_out=sums[:, j : j + 1],
                )
                nc.scalar.activation(
                    out=ysq[:], in_=ycol,
                    func=mybir.ActivationFunctionType.Square,
                    accum_out=sqs[:, j : j + 1],
                )
                off += Lj

            # combine band partials -> [128, 2]
            stats = spool.tile([P, 2], f32, tag="stats")
            nc.vector.reduce_sum(out=stats[:, 0:1], in_=sums[:],
                                 axis=mybir.AxisListType.X)
            nc.vector.reduce_sum(out=stats[:, 1:2], in_=sqs[:],
                                 axis=mybir.AxisListType.X)

            # ---- AllReduce stats over the 8 cores ----
            if no_cc:
                gstats = stats
            else:
                cc_in = dram.tile([P, 2], f32, tag="ccin")
                cc_out = dram.tile([P, 2], f32, tag="ccout")
                nc.gpsimd.dma_start(out=cc_in[:], in_=stats[:])
                nc.gpsimd.collective_compute(
                    "AllReduce",
                    mybir.AluOpType.add,
                    replica_groups=[list(range(N_CORES))],
                    ins=[cc_in.opt()],
                    outs=[cc_out.opt()],
                )
                gstats = spool.tile([P, 2], f32, tag="gstats")
                nc.gpsimd.dma_start(out=gstats[:], in_=cc_out[:])

            # ---- BN scale/shift ----
            me = spool.tile([P, 2], f32, tag="me")
            nc.scalar.mul(out=me[:], in_=gstats[:], mul=1.0 / B)
            mean = me[:, 0:1]
            var = spool.tile([P, 1], f32, tag="var")
            nc.vector.tensor_mul(out=var[:], in0=mean, in1=mean)
            nc.vector.tensor_sub(out=var[:], in0=me[:, 1:2], in1=var[:])
            std = spool.tile([P, 1], f32, tag="std")
            nc.scalar.activation(
                out=std[:], in_=var[:],
                func=mybir.ActivationFunctionType.Sqrt,
                bias=eps_sb[:])
            rstd = spool.tile([P, 1], f32, tag="rstd")
            nc.vector.reciprocal(out=rstd[:], in_=std[:])
            scl = spool.tile([P, 1], f32, tag="scl")
            nc.vector.tensor_mul(out=scl[:], in0=gm_sb[:], in1=rstd[:])
            shf = spool.tile([P, 1], f32, tag="shf")
            nc.vector.tensor_mul(out=shf[:], in0=mean, in1=scl[:])
            nc.vector.tensor_sub(out=shf[:], in0=bt_sb[:], in1=shf[:])

            # ---- normalize + relu (per band), transpose back, one store ----
            o_sb = ofast.tile([P, N_BANDS, C_OUT], f32, tag="o")
            for j in range(N_BANDS):
                yf = opool.tile([C_OUT, P], f32, tag="yf")
                nc.scalar.activation(
                    out=yf[:], in_=y_sb[:, j * P : (j + 1) * P],
                    func=mybir.ActivationFunctionType.Relu,
                    bias=shf[:], scale=scl[:],
                )
                tp2 = ps_tp2.tile([P, P], f32, tag="tp2")
                nc.tensor.transpose(tp2[:], yf[:], ident[:])
                nc.vector.tensor_copy(o_sb[:, j, :], tp2[:])
            # out[j*P + p, c] <- o_sb[p, j, c]; two DMAs so the first half's
            # transfer overlaps the second half's transposes/copies
            out_view = out.rearrange("(j p) c -> p j c", p=P)
            nc.sync.dma_start(out=out_view[:, 0:2, :], in_=o_sb[:, 0:2, :])
            nc.sync.dma_start(out=out_view[:, 2:4, :], in_=o_sb[:, 2:4, :])

        if repeat > 1:
            with tc.For_i(0, repeat, 1):
                for _ in range(unroll):
                    body()
        else:
            for _ in range(unroll):
                body()

    nc.compile()
    return nc


def _layout(length):
    """Global sort -> band lengths (ceil to 8), per-(core,band) segment ids."""
    length = np.asarray(length, np.int64)
    starts = np.zeros(B, np.int64)
    starts[1:] = np.cumsum(length)[:-1]
    order = np.argsort(-length, kind="stable")
    band = N_CORES * P
    # multiple of 8 so each chunk supports 3 clean halvings
    Ls = [-(-int(length[order[band * j]]) // 8) * 8 for j in range(N_BANDS)]
    # seg_ids[c, j, p] = original segment id handled by core c, band j, row p
    seg_ids = np.empty((N_CORES, N_BANDS, P), np.int64)
    for j in range(N_BANDS):
        for c in range(N_CORES):
            seg_ids[c, j] = order[band * j + P * c : band * j + P * (c + 1)]
    return starts, Ls, seg_ids


def _pack_inputs(x, length, conv_w, conv_b, gamma, beta, starts, Ls, seg_ids):
    """Pack x into the tree-slot-major chunked row layout (see module doc).

    Row (c, j, p) = concat over chunks k of arr8[:, :, g0:g1].ravel() where
    arr8 = padded [32, Lj] -> reshape [32, G, 8] -> transpose to [8, 32, G].
    """
    Ltot = int(sum(Ls))
    xp = np.empty((N_CORES, P, 32 * Ltot), np.float16)
    offs = np.concatenate([[0], np.cumsum(Ls)]).astype(np.int64)
    length = np.asarray(length, np.int64)
    x = np.asarray(x, np.float32)
    pad = np.empty((32,), np.float16)
    for c in range(N_CORES):
        for j in range(N_BANDS):
            Lj = int(Ls[j])
            G = Lj // 8
            sizes = _chunk_sizes(G)
            bounds = np.concatenate([[0], np.cumsum(sizes)])
            base = 32 * int(offs[j])
            buf = np.full((P, 32, Lj), FMIN, np.float16)
            for p in range(P):
                s = int(starts[seg_ids[c, j, p]])
                l = int(length[seg_ids[c, j, p]])
                buf[p, :, :l] = x[s : s + l].T
            # [P, 32, G, 8] -> [P, 8, 32, G] (slot-major)
            arr8 = buf.reshape(P, 32, G, 8).transpose(0, 3, 1, 2)
            pos = base
            for k in range(len(sizes)):
                g0, g1 = int(bounds[k]), int(bounds[k + 1])
                E = 256 * (g1 - g0)
                xp[c, :, pos : pos + E] = arr8[:, :, :, g0:g1].reshape(P, -1)
                pos += E
    wt = np.ascontiguousarray(np.asarray(conv_w, np.float32).T)  # [32, 128]
    cb = np.ascontiguousarray(conv_b.reshape(C_OUT, 1), np.float32)
    gm = np.ascontiguousarray(gamma.reshape(C_OUT, 1), np.float32)
    bt = np.ascontiguousarray(beta.reshape(C_OUT, 1), np.float32)
    in_maps = [
        {"xp": xp[c], "wt": wt, "cb": cb, "gm": gm, "bt": bt}
        for c in range(N_CORES)
    ]
    return in_maps


def _run(x, length, conv_w, conv_b, gamma, beta, trace=False):
    from concourse.bass_utils import run_bass_kernel_spmd

    x = np.asarray(x, np.float32)
    length = np.asarray(length)
    assert x.shape == (N, C_IN) and length.shape == (B,)

    starts, Ls, seg_ids = _layout(length)
    in_maps = _pack_inputs(
        x, length, np.asarray(conv_w), np.asarray(conv_b),
        np.asarray(gamma), np.asarray(beta), starts, Ls, seg_ids,
    )

    key = tuple(Ls)
    if key not in _prog_cache:
        _prog_cache[key] = _build_program(Ls)
    nc = _prog_cache[key]

    res = run_bass_kernel_spmd(nc, in_maps, list(range(N_CORES)), trace=trace)

    full = np.empty((B, C_OUT), np.float32)
    for c in range(N_CORES):
        full[seg_ids[c].reshape(-1)] = res.results[c]["out"]
    return full, res


def kernel(x, length, conv_w, conv_b, gamma, beta):
    full, _ = _run(x, length, conv_w, conv_b, gamma, beta, trace=False)
    return full
